# revision 4
# baseline (speedup 1.0000x reference)
"""Trainium2 Bass kernel for a segmented tensor-product contraction.

Computation (per batch row z, channel u, segments of width U=128):
  out[z, so, u] += c_p * x0[i0[z], s0_p, u] * prod_k x1[z, sk_p, u]
for 256 paths of degree 1..3 over S=16 segments.

Strategy:
  - Data-parallel over z across 8 NeuronCores (512 rows each).
  - On-chip layout: [u (partitions) x z (free dim)] per segment; every
    elementwise op is a [128, 512] instruction.
  - x0 row gather: host builds one-hot(i0) per core; TensorEngine computes
    x0gT[s] = x0[:, s]^T @ onehot (gather + transpose for free).
  - Factorization (globally optimized per so-group): suffix products
    sg(s0,s) = x0g[s0]*x1[s] and pairs pr(a,b) = x1[a]*x1[b]; each path is
    one tensor_tensor plus a coefficient scale on ScalarE (or a fused
    scalar_tensor_tensor on VectorE for a fraction of paths).
  - Product builds are packed into merged multi-segment instructions
    (sg runs share one instr via a stride-0 broadcast of x0g[s0]; pair
    runs along constant delta read contiguous x1 spans).
  - Output accumulation on TensorEngine: identity-matmul each path term
    into a per-segment PSUM bank (exact f32 adds). 16 output segments ->
    two groups of 8 banks; the so-partition is optimized to minimize
    duplicated product builds.
"""

import os
from collections import defaultdict

import numpy as np

U = 128
S = 16
NELEM = 64
Z = 4096
NCORES = 8
ZS = Z // NCORES  # 512 rows per core

LAST_EXEC_NS = None
LAST_RESULTS = None

F32 = "float32"


def _parse_paths(idxs, coeffs):
    paths = []  # (degree, x1segs_sorted, s0, so, coeff)
    for idx, cf in zip(idxs, coeffs):
        d = idx.shape[1] - 2
        for r, c in zip(idx, cf):
            r = [int(v) for v in r]
            paths.append((d, tuple(sorted(r[:d])), r[d], r[d + 1], float(c)))
    return paths


def _options(p):
    """Candidate (products, form) decompositions for a path.

    Each option: (frozenset of product keys, form)
    form = (in0_ref, in1_ref) with refs ('x1',s) ('x0g',s) ('sg',(s0,s))
    ('pair',(a,b)); d1 form = (('sg',(s0,s)), None).
    Product keys: ('sg',(s0,s)), ('pair',(a,b)).
    """
    d, segs, s0, so, c = p
    if d == 1:
        k = ("sg", (s0, segs[0]))
        return [(frozenset([k]), (k, None))]
    if d == 2:
        a, b = segs
        opts = [
            (frozenset([("sg", (s0, b))]), (("x1", a), ("sg", (s0, b)))),
            (frozenset([("sg", (s0, a))]), (("x1", b), ("sg", (s0, a)))),
            (frozenset([("pair", (a, b))]), (("pair", (a, b)), ("x0g", s0))),
        ]
        return opts
    a, b, cc = segs
    return [
        (
            frozenset([("pair", (a, b)), ("sg", (s0, cc))]),
            (("pair", (a, b)), ("sg", (s0, cc))),
        ),
        (
            frozenset([("pair", (a, cc)), ("sg", (s0, b))]),
            (("pair", (a, cc)), ("sg", (s0, b))),
        ),
        (
            frozenset([("pair", (b, cc)), ("sg", (s0, a))]),
            (("pair", (b, cc)), ("sg", (s0, a))),
        ),
    ]


def _optimize_group(gpaths, n_sweeps=4):
    """Choose per-path decomposition minimizing total unique products."""
    choices = [0] * len(gpaths)
    opts = [_options(p) for p in gpaths]
    for _ in range(n_sweeps):
        counts = defaultdict(int)
        for i, p in enumerate(gpaths):
            for k in opts[i][choices[i]][0]:
                counts[k] += 1
        changed = False
        for i, p in enumerate(gpaths):
            best, best_cost = choices[i], None
            for j, (prods, _) in enumerate(opts[i]):
                # marginal cost: products not used by anyone else
                cost = 0.0
                for k in prods:
                    others = counts[k] - (1 if k in opts[i][choices[i]][0] else 0)
                    cost += 1.0 / (1 + others)
                if best_cost is None or cost < best_cost - 1e-9:
                    best, best_cost = j, cost
            if best != choices[i]:
                # update counts incrementally
                for k in opts[i][choices[i]][0]:
                    counts[k] -= 1
                for k in opts[i][best][0]:
                    counts[k] += 1
                choices[i] = best
                changed = True
        if not changed:
            break
    products = set()
    forms = []
    for i, p in enumerate(gpaths):
        prods, form = opts[i][choices[i]]
        products |= prods
        forms.append(form)
    return products, forms


def _group_cost(paths, sos_a):
    """Estimate total builds for a candidate so-partition."""
    total = 0
    for sos in (sos_a, [s for s in range(S) if s not in sos_a]):
        gp = [p for p in paths if p[3] in sos]
        prods, _ = _optimize_group(gp, n_sweeps=4)
        total += len(prods)
    return total


def _optimize_partition(paths):
    """Two-stage exhaustive search of the 8/8 so-partition (C(16,8)/2 =
    6435 candidates): cheap 1-sweep proxy scan, then exact re-scoring of
    the best candidates."""
    from itertools import combinations

    def cost(sos_a, sweeps):
        total = 0
        for sos in (sos_a, [s for s in range(S) if s not in sos_a]):
            gp = [p for p in paths if p[3] in sos]
            prods, _ = _optimize_group(gp, n_sweeps=sweeps)
            total += len(prods)
        return total

    cands = [c for c in combinations(range(S), 8) if 0 in c]
    scored = sorted(cands, key=lambda c: cost(list(c), 1))[:30]
    best = min(scored, key=lambda c: cost(list(c), 4))
    cur = list(best)
    other = [s for s in range(S) if s not in cur]
    return cur, other


def _plan_merges(products):
    """Pack product builds into merged instructions.

    Returns (slot_of, builds) where slot_of maps product key -> slot index
    and builds is a list of ('sg_run', s0, s_lo, n, slot_lo) or
    ('pair_run', delta, a_lo, n, slot_lo).
    """
    slot_of = {}
    builds = []
    next_slot = 0
    sgs = defaultdict(list)  # s0 -> sorted s list
    prs = defaultdict(list)  # delta -> sorted a list
    for k in products:
        if k[0] == "sg":
            sgs[k[1][0]].append(k[1][1])
        else:
            a, b = k[1]
            prs[b - a].append(a)
    squares = sorted(prs.pop(0, []))
    # pair runs first: they depend only on x1t (no gather chain), so the
    # DVE can start on them while the x0 gather pipeline fills
    for delta in sorted(prs):
        aa = sorted(prs[delta])
        run = [aa[0]]
        for a in aa[1:] + [None]:
            if a is not None and a == run[-1] + 1:
                run.append(a)
            else:
                kind = "pair_run"
                builds.append((kind, delta, run[0], len(run), next_slot))
                for i, ra in enumerate(run):
                    slot_of[("pair", (ra, ra + delta))] = next_slot + i
                next_slot += len(run)
                if a is not None:
                    run = [a]
    for s0 in sorted(sgs):
        ss = sorted(sgs[s0])
        run = [ss[0]]
        for s in ss[1:] + [None]:
            if s is not None and s == run[-1] + 1:
                run.append(s)
            else:
                builds.append(("sg_run", s0, run[0], len(run), next_slot))
                for i, rs in enumerate(run):
                    slot_of[("sg", (s0, rs))] = next_slot + i
                next_slot += len(run)
                if s is not None:
                    run = [s]
    return slot_of, builds, next_slot, squares


def _build_plan(idxs, coeffs):
    """Full schedule. Returns (groups, all_sq).

    Joint factorization over ALL paths; products used by both so-groups
    are built once into a shared slot region and stay resident across
    both PSUM phases. Group-unique products overlay one reuse region.
    """
    paths = _parse_paths(idxs, coeffs)
    products, forms = _optimize_group(paths, n_sweeps=6)
    part_a = list(range(8))
    part_b = list(range(8, 16))

    all_sq = sorted(
        set(k[1][0] for k in products if k[0] == "pair" and k[1][0] == k[1][1])
    )
    sq_keys = set(("pair", (s, s)) for s in all_sq)

    # classify products by which groups use them
    use_a, use_b = set(), set()
    for p, form in zip(paths, forms):
        tgt = use_a if p[3] in part_a else use_b
        for r in form:
            if r and r[0] in ("sg", "pair") and r not in sq_keys:
                tgt.add(r)
    shared = use_a & use_b
    uniq = {0: use_a - shared, 1: use_b - shared}

    slot_shared, builds_shared, ns, _ = _plan_merges(shared)
    slot_a, builds_a, na, _ = _plan_merges(uniq[0])
    slot_b, builds_b, nb, _ = _plan_merges(uniq[1])
    base2 = ns
    n_main = ns + max(na, nb)
    sq_slot = {s: n_main + i for i, s in enumerate(all_sq)}
    n_slots = n_main + len(all_sq)

    def shift(builds, slot, delta):
        bs = [(b[0], b[1], b[2], b[3], b[4] + delta) for b in builds]
        sl = {k: v + delta for k, v in slot.items()}
        return bs, sl

    builds_a, slot_a = shift(builds_a, slot_a, base2)
    builds_b, slot_b = shift(builds_b, slot_b, base2)

    groups = []
    for gi, (sos, gbuilds, gslot) in enumerate(
        (
            (part_a, builds_shared + builds_a, {**slot_shared, **slot_a}),
            (part_b, builds_b, {**slot_shared, **slot_b}),
        )
    ):
        slot_of = dict(gslot)
        for s in all_sq:
            slot_of[("pair", (s, s))] = sq_slot[s]
        gidx = [i for i, p in enumerate(paths) if p[3] in sos]
        order = sorted(
            gidx,
            key=lambda i: (
                paths[i][0] != 1,
                max(
                    (
                        slot_of[r]
                        for r in forms[i]
                        if r and r[0] in ("sg", "pair")
                    ),
                    default=-1,
                ),
            ),
        )
        path_ops = [
            (paths[i][0], forms[i][0], forms[i][1], paths[i][4], paths[i][3])
            for i in order
        ]
        groups.append(
            dict(
                sos=sos,
                builds=gbuilds,
                slot_of=slot_of,
                n_slots=n_slots,
                path_ops=path_ops,
            )
        )
    return groups, all_sq


SLAB = 32  # coefficient-diagonal matrices per DMA slab


def _build_bass(groups, dtype_name, act_frac, warmup, pool_frac=0.0, all_sq=(), gpsimd_every=0):
    import concourse.bacc as bacc
    import concourse.mybir as mybir
    from concourse.tile import TileContext

    dt = mybir.dt.float32 if dtype_name == F32 else mybir.dt.bfloat16
    MULT = mybir.AluOpType.mult

    nc = bacc.Bacc("TRN2", debug=False)

    n_paths_total = sum(len(g["path_ops"]) for g in groups)
    n_slabs = (n_paths_total + SLAB - 1) // SLAB

    x1t_d = nc.dram_tensor("x1t", [S * U, ZS], dt, kind="ExternalInput")
    x0_d = nc.dram_tensor("x0w", [NELEM, S * U], dt, kind="ExternalInput")
    oh_d = nc.dram_tensor("oh", [NELEM, ZS], dt, kind="ExternalInput")
    cd_d = nc.dram_tensor("cdiag", [n_slabs * SLAB * U, U], dt, kind="ExternalInput")
    out_d = nc.dram_tensor("outt", [S * U, ZS], dt, kind="ExternalOutput")
    junk_d = nc.dram_tensor("junk", [U, ZS], mybir.dt.float32)

    max_slots = max(g["n_slots"] for g in groups)
    coeff_order = []  # flat list of coefficients in emission order

    with TileContext(nc) as tc:
        with tc.tile_pool(name="persist", bufs=1) as persist, tc.tile_pool(
            name="tmp", bufs=16
        ) as tmp_pool, tc.tile_pool(name="slab", bufs=2) as slab_pool:
            x1t = persist.tile([U, S * ZS], dt, tag="x1t")
            x0g = persist.tile([U, S * ZS], dt, tag="x0g")
            out_sb = persist.tile([U, S * ZS], dt, tag="out")
            prod = persist.tile([U, max_slots * ZS], dt, tag="prod")
            x0_sb = persist.tile([NELEM, S * U], dt, tag="x0w")
            oh_sb = persist.tile([NELEM, ZS], dt, tag="oh")

            def seg(t, s):
                return t[:, s * ZS : (s + 1) * ZS]

            def span(t, lo, n):
                return t[:, lo * ZS : (lo + n) * ZS]

            nc.sync.dma_start(out=oh_sb[:], in_=oh_d[:])
            nc.sync.dma_start(out=x0_sb[:], in_=x0_d[:])
            # x1t segments in order of first use by the build schedule
            seg_order = []

            def _want(s):
                if s not in seg_order:
                    seg_order.append(s)

            for s in all_sq:
                _want(s)
            for g in groups:
                for b in g["builds"]:
                    kind, key, lo, n, _ = b
                    if kind == "pair_run":
                        for i in range(n):
                            _want(lo + i)
                            _want(lo + i + key)
                    else:
                        for i in range(n):
                            _want(lo + i)
            for s in range(S):
                _want(s)
            for s in seg_order:
                nc.sync.dma_start(out=seg(x1t, s), in_=x1t_d[s * U : (s + 1) * U, :])

            # global square products on ScalarE (before any ACT Copy use,
            # emitted as consecutive runs to avoid table-set thrashing)
            if all_sq:
                max_g = groups[0]["n_slots"] - len(all_sq)
                run = [all_sq[0]]
                ri = 0
                for s in list(all_sq[1:]) + [None]:
                    if s is not None and s == run[-1] + 1:
                        run.append(s)
                    else:
                        nc.scalar.activation(
                            span(prod, max_g + ri, len(run)),
                            span(x1t, run[0], len(run)),
                            mybir.ActivationFunctionType.Square,
                        )
                        ri += len(run)
                        if s is not None:
                            run = [s]

            # PE warmup burst + gather matmuls
            with tc.tile_pool(name="gpsum", bufs=4, space="PSUM") as gpsum:
                if warmup > 0:
                    wt = gpsum.tile([U, ZS], mybir.dt.float32, tag="warm", bufs=1)
                    for i in range(warmup):
                        nc.tensor.matmul(
                            wt[:],
                            x0_sb[:, 0:U],
                            oh_sb[:],
                            start=(i == 0),
                            stop=(i == warmup - 1),
                        )
                    ws = tmp_pool.tile([U, ZS], mybir.dt.float32, tag="warms", bufs=1)
                    nc.scalar.copy(out=ws[:], in_=wt[:])
                    nc.sync.dma_start(out=junk_d[:], in_=ws[:])
                for s in range(S):
                    pt = gpsum.tile([U, ZS], mybir.dt.float32, tag="gps")
                    nc.tensor.matmul(
                        pt[:],
                        x0_sb[:, s * U : (s + 1) * U],
                        oh_sb[:],
                        start=True,
                        stop=True,
                    )
                    nc.scalar.copy(out=seg(x0g, s), in_=pt[:])

            slab_state = {"idx": -1, "tile": None, "fin": 0}
            for g in groups:
                sos, builds, slot_of, path_ops = (
                    g["sos"],
                    g["builds"],
                    g["slot_of"],
                    g["path_ops"],
                )
                # interleave: emit builds, releasing paths when ready
                ready_after = defaultdict(list)  # build idx -> path indices
                path_needs = []
                for i, (d, r1, r2, c, so) in enumerate(path_ops):
                    needs = set()
                    for r in (r1, r2):
                        if r and r[0] in ("sg", "pair"):
                            needs.add(slot_of[r])
                    path_needs.append(needs)
                slot_done_at = {}
                for bi, b in enumerate(builds):
                    for i in range(b[3]):
                        slot_done_at[b[4] + i] = bi
                for i, needs in enumerate(path_needs):
                    bi = max(
                        (slot_done_at.get(s, -1) for s in needs), default=-1
                    )
                    ready_after[bi].append(i)

                # emission order determines PE program order: derive
                # first/last per so from it for the start/stop flags
                emit_order = list(ready_after[-1])
                for bi in range(len(builds)):
                    emit_order.extend(ready_after[bi])
                first_for_so = {}
                last_for_so = {}
                for i in emit_order:
                    so = path_ops[i][4]
                    if so not in first_for_so:
                        first_for_so[so] = i
                    last_for_so[so] = i

                acc = {}
                with tc.tile_pool(
                    name=f"acc{sos[0]}", bufs=8, space="PSUM"
                ) as acc_pool:
                    for so in sos:
                        if so in first_for_so:
                            acc[so] = acc_pool.tile(
                                [U, ZS],
                                mybir.dt.float32,
                                tag=f"acc{sos.index(so)}",
                                name=f"acc_{so}",
                                bufs=1,
                            )

                    def pref(r):
                        kind, key = r
                        if kind == "x1":
                            return seg(x1t, key)
                        if kind == "x0g":
                            return seg(x0g, key)
                        sl = slot_of[r]
                        return seg(prod, sl)

                    def emit_path(i):
                        d, r1, r2, c, so = path_ops[i]
                        gi = len(coeff_order)
                        coeff_order.append(c)
                        sj, sk = gi // SLAB, gi % SLAB
                        if slab_state["idx"] != sj:
                            slab_state["idx"] = sj
                            st = slab_pool.tile(
                                [U, SLAB * U], dt, tag="slab", name=f"slab{sj}"
                            )
                            slab_state["tile"] = st
                            nc.sync.dma_start(
                                out=st[:].rearrange("p (d c) -> p d c", d=SLAB),
                                in_=cd_d[sj * SLAB * U : (sj + 1) * SLAB * U, :]
                                .rearrange("(d p) c -> p d c", p=U),
                            )
                        st = slab_state["tile"]
                        if d == 1:
                            rhs = pref(r1)
                        else:
                            t1 = tmp_pool.tile([U, ZS], dt, tag="tmp", name=f"t1{i}")
                            # round-robin a fraction of the final products onto
                            # the GpSimd engine to relieve the saturated DVE
                            eng = nc.vector
                            if gpsimd_every > 0:
                                slab_state["fin"] += 1
                                if slab_state["fin"] % gpsimd_every == 0:
                                    eng = nc.gpsimd
                            eng.tensor_tensor(
                                out=t1[:], in0=pref(r1), in1=pref(r2), op=MULT
                            )
                            rhs = t1[:]
                        nc.tensor.matmul(
                            acc[so][:],
                            st[:, sk * U : (sk + 1) * U],
                            rhs,
                            start=(i == first_for_so[so]),
                            stop=(i == last_for_so[so]),
                        )

                    for i in ready_after[-1]:
                        emit_path(i)
                    for bi, b in enumerate(builds):
                        kind = b[0]
                        if kind == "sq_run":
                            _, delta, a_lo, n, slot_lo = b
                            nc.scalar.activation(
                                span(prod, slot_lo, n),
                                span(x1t, a_lo, n),
                                mybir.ActivationFunctionType.Square,
                            )
                        elif kind == "sg_run":
                            _, s0, s_lo, n, slot_lo = b
                            in0 = (
                                seg(x0g, s0)
                                .rearrange("p (o z) -> p o z", o=1)
                                .broadcast_to([U, n, ZS])
                            )
                            in1 = span(x1t, s_lo, n).rearrange(
                                "p (r z) -> p r z", r=n
                            )
                            out = span(prod, slot_lo, n).rearrange(
                                "p (r z) -> p r z", r=n
                            )
                            nc.vector.tensor_tensor(
                                out=out, in0=in0, in1=in1, op=MULT
                            )
                        else:
                            _, delta, a_lo, n, slot_lo = b
                            in0 = span(x1t, a_lo, n).rearrange(
                                "p (r z) -> p r z", r=n
                            )
                            in1 = span(x1t, a_lo + delta, n).rearrange(
                                "p (r z) -> p r z", r=n
                            )
                            out = span(prod, slot_lo, n).rearrange(
                                "p (r z) -> p r z", r=n
                            )
                            nc.vector.tensor_tensor(
                                out=out, in0=in0, in1=in1, op=MULT
                            )
                        for i in ready_after[bi]:
                            emit_path(i)

                    for so in sos:
                        if so in acc:
                            nc.scalar.copy(out=seg(out_sb, so), in_=acc[so][:])
                        else:
                            nc.vector.memset(seg(out_sb, so), 0.0)

            for s in range(S):
                nc.sync.dma_start(out=out_d[s * U : (s + 1) * U, :], in_=seg(out_sb, s))

    nc.compile()
    return nc, coeff_order


def kernel(x0, x1, coeff1, coeff2, coeff3, i0, idx1, idx2, idx3):
    global LAST_EXEC_NS, LAST_RESULTS
    from concourse.bass_utils import run_bass_kernel_spmd

    x0 = np.asarray(x0, dtype=np.float32)
    x1 = np.asarray(x1, dtype=np.float32)
    i0 = np.asarray(i0).astype(np.int64)
    idxs = [np.asarray(a) for a in (idx1, idx2, idx3)]
    coeffs = [np.asarray(c, dtype=np.float32) for c in (coeff1, coeff2, coeff3)]

    dtype_name = os.environ.get("KERNEL_DTYPE", "bfloat16")
    act_frac = float(os.environ.get("KERNEL_ACT_FRAC", "0.55"))
    pool_frac = float(os.environ.get("KERNEL_POOL_FRAC", "0.3"))
    warmup = int(os.environ.get("KERNEL_WARMUP", "12"))
    gpsimd_every = int(os.environ.get("KERNEL_GPSIMD_EVERY", "3"))
    npdt = np.float32
    if dtype_name != F32:
        import ml_dtypes

        npdt = ml_dtypes.bfloat16

    groups, all_sq = _build_plan(idxs, coeffs)
    nc, coeff_order = _build_bass(groups, dtype_name, act_frac, warmup, pool_frac, all_sq, gpsimd_every)
    n_slabs = (len(coeff_order) + SLAB - 1) // SLAB
    cdiag = np.zeros((n_slabs * SLAB * U, U), dtype=npdt)
    for gi, c in enumerate(coeff_order):
        blk = cdiag[gi * U : (gi + 1) * U, :]
        np.fill_diagonal(blk, np.asarray(c, dtype=npdt))

    in_maps = []
    eye = np.arange(NELEM)
    x0c = x0.astype(npdt)
    for c in range(NCORES):
        zl, zh = c * ZS, (c + 1) * ZS
        shard = x1[zl:zh]
        x1t = np.ascontiguousarray(
            shard.reshape(ZS, S, U).transpose(1, 2, 0).reshape(S * U, ZS)
        ).astype(npdt)
        oh = (i0[zl:zh][None, :] == eye[:, None]).astype(npdt)
        in_maps.append({"x1t": x1t, "x0w": x0c, "oh": oh, "cdiag": cdiag})

    trace = os.environ.get("BASS_TRACE", "") not in ("", "0")
    trace_cores = None
    tc_env = os.environ.get("KERNEL_TRACE_CORES", "")
    if tc_env:
        trace_cores = [int(x) for x in tc_env.split(",")]
    res = run_bass_kernel_spmd(
        nc, in_maps, core_ids=list(range(NCORES)), trace=trace,
        trace_cores=trace_cores,
    )
    LAST_EXEC_NS = res.exec_time_ns
    LAST_RESULTS = res

    out = np.empty((Z, S * U), dtype=np.float32)
    for c in range(NCORES):
        outt = np.asarray(res.results[c]["outt"], dtype=np.float32)
        out[c * ZS : (c + 1) * ZS] = (
            outt.reshape(S, U, ZS).transpose(2, 0, 1).reshape(ZS, S * U)
        )
    return out



# revision 8
# speedup vs baseline: 1.3254x; 1.3254x over previous
"""Trainium2 Bass kernel for a segmented tensor-product contraction.

Computation (per batch row z, channel u, segments of width U=128):
  out[z, so, u] += c_p * x0[i0[z], s0_p, u] * prod_k x1[z, sk_p, u]
for 256 paths of degree 1..3 over S=16 segments.

Strategy:
  - Data-parallel over z across 8 NeuronCores (512 rows each).
  - On-chip layout: [u (partitions) x z (free dim)] per segment; every
    elementwise op is a [128, 512] instruction.
  - x0 row gather: host builds one-hot(i0) per core; TensorEngine computes
    x0gT[s] = x0[:, s]^T @ onehot (gather + transpose for free).
  - Factorization (globally optimized per so-group): suffix products
    sg(s0,s) = x0g[s0]*x1[s] and pairs pr(a,b) = x1[a]*x1[b]; each path is
    one tensor_tensor plus a coefficient scale on ScalarE (or a fused
    scalar_tensor_tensor on VectorE for a fraction of paths).
  - Product builds are packed into merged multi-segment instructions
    (sg runs share one instr via a stride-0 broadcast of x0g[s0]; pair
    runs along constant delta read contiguous x1 spans).
  - Output accumulation on TensorEngine: identity-matmul each path term
    into a per-segment PSUM bank (exact f32 adds). 16 output segments ->
    two groups of 8 banks; the so-partition is optimized to minimize
    duplicated product builds.
"""

import os
from collections import defaultdict

import numpy as np

U = 128
S = 16
NELEM = 64
Z = 4096
NCORES = 8
ZS = Z // NCORES  # 512 rows per core

LAST_EXEC_NS = None
LAST_RESULTS = None

F32 = "float32"


def _parse_paths(idxs, coeffs):
    paths = []  # (degree, x1segs_sorted, s0, so, coeff)
    for idx, cf in zip(idxs, coeffs):
        d = idx.shape[1] - 2
        for r, c in zip(idx, cf):
            r = [int(v) for v in r]
            paths.append((d, tuple(sorted(r[:d])), r[d], r[d + 1], float(c)))
    return paths


def _options(p):
    """Candidate (products, form) decompositions for a path.

    Each option: (frozenset of product keys, form)
    form = (in0_ref, in1_ref) with refs ('x1',s) ('x0g',s) ('sg',(s0,s))
    ('pair',(a,b)); d1 form = (('sg',(s0,s)), None).
    Product keys: ('sg',(s0,s)), ('pair',(a,b)).
    """
    d, segs, s0, so, c = p
    if d == 1:
        k = ("sg", (s0, segs[0]))
        return [(frozenset([k]), (k, None))]
    if d == 2:
        a, b = segs
        opts = [
            (frozenset([("sg", (s0, b))]), (("x1", a), ("sg", (s0, b)))),
            (frozenset([("sg", (s0, a))]), (("x1", b), ("sg", (s0, a)))),
            (frozenset([("pair", (a, b))]), (("pair", (a, b)), ("x0g", s0))),
        ]
        return opts
    a, b, cc = segs
    return [
        (
            frozenset([("pair", (a, b)), ("sg", (s0, cc))]),
            (("pair", (a, b)), ("sg", (s0, cc))),
        ),
        (
            frozenset([("pair", (a, cc)), ("sg", (s0, b))]),
            (("pair", (a, cc)), ("sg", (s0, b))),
        ),
        (
            frozenset([("pair", (b, cc)), ("sg", (s0, a))]),
            (("pair", (b, cc)), ("sg", (s0, a))),
        ),
    ]


def _optimize_group(gpaths, n_sweeps=4):
    """Choose per-path decomposition minimizing total unique products."""
    choices = [0] * len(gpaths)
    opts = [_options(p) for p in gpaths]
    for _ in range(n_sweeps):
        counts = defaultdict(int)
        for i, p in enumerate(gpaths):
            for k in opts[i][choices[i]][0]:
                counts[k] += 1
        changed = False
        for i, p in enumerate(gpaths):
            best, best_cost = choices[i], None
            for j, (prods, _) in enumerate(opts[i]):
                # marginal cost: products not used by anyone else
                cost = 0.0
                for k in prods:
                    others = counts[k] - (1 if k in opts[i][choices[i]][0] else 0)
                    cost += 1.0 / (1 + others)
                if best_cost is None or cost < best_cost - 1e-9:
                    best, best_cost = j, cost
            if best != choices[i]:
                # update counts incrementally
                for k in opts[i][choices[i]][0]:
                    counts[k] -= 1
                for k in opts[i][best][0]:
                    counts[k] += 1
                choices[i] = best
                changed = True
        if not changed:
            break
    products = set()
    forms = []
    for i, p in enumerate(gpaths):
        prods, form = opts[i][choices[i]]
        products |= prods
        forms.append(form)
    return products, forms


def _group_cost(paths, sos_a):
    """Estimate total builds for a candidate so-partition."""
    total = 0
    for sos in (sos_a, [s for s in range(S) if s not in sos_a]):
        gp = [p for p in paths if p[3] in sos]
        prods, _ = _optimize_group(gp, n_sweeps=4)
        total += len(prods)
    return total


def _optimize_partition(paths):
    """Two-stage exhaustive search of the 8/8 so-partition (C(16,8)/2 =
    6435 candidates): cheap 1-sweep proxy scan, then exact re-scoring of
    the best candidates."""
    from itertools import combinations

    def cost(sos_a, sweeps):
        total = 0
        for sos in (sos_a, [s for s in range(S) if s not in sos_a]):
            gp = [p for p in paths if p[3] in sos]
            prods, _ = _optimize_group(gp, n_sweeps=sweeps)
            total += len(prods)
        return total

    cands = [c for c in combinations(range(S), 8) if 0 in c]
    scored = sorted(cands, key=lambda c: cost(list(c), 1))[:30]
    best = min(scored, key=lambda c: cost(list(c), 4))
    cur = list(best)
    other = [s for s in range(S) if s not in cur]
    return cur, other


def _plan_merges(products):
    """Pack product builds into merged instructions.

    Returns (slot_of, builds) where slot_of maps product key -> slot index
    and builds is a list of ('sg_run', s0, s_lo, n, slot_lo) or
    ('pair_run', delta, a_lo, n, slot_lo).
    """
    slot_of = {}
    builds = []
    next_slot = 0
    sgs = defaultdict(list)  # s0 -> sorted s list
    prs = defaultdict(list)  # delta -> sorted a list
    for k in products:
        if k[0] == "sg":
            sgs[k[1][0]].append(k[1][1])
        else:
            a, b = k[1]
            prs[b - a].append(a)
    squares = sorted(prs.pop(0, []))
    # pair runs first: they depend only on x1t (no gather chain), so the
    # DVE can start on them while the x0 gather pipeline fills
    for delta in sorted(prs):
        aa = sorted(prs[delta])
        run = [aa[0]]
        for a in aa[1:] + [None]:
            if a is not None and a == run[-1] + 1:
                run.append(a)
            else:
                kind = "pair_run"
                builds.append((kind, delta, run[0], len(run), next_slot))
                for i, ra in enumerate(run):
                    slot_of[("pair", (ra, ra + delta))] = next_slot + i
                next_slot += len(run)
                if a is not None:
                    run = [a]
    for s0 in sorted(sgs):
        ss = sorted(sgs[s0])
        run = [ss[0]]
        for s in ss[1:] + [None]:
            if s is not None and s == run[-1] + 1:
                run.append(s)
            else:
                builds.append(("sg_run", s0, run[0], len(run), next_slot))
                for i, rs in enumerate(run):
                    slot_of[("sg", (s0, rs))] = next_slot + i
                next_slot += len(run)
                if s is not None:
                    run = [s]
    return slot_of, builds, next_slot, squares


def _build_plan(idxs, coeffs):
    """Full schedule. Returns (groups, all_sq).

    Joint factorization over ALL paths; products used by both so-groups
    are built once into a shared slot region and stay resident across
    both PSUM phases. Group-unique products overlay one reuse region.
    """
    paths = _parse_paths(idxs, coeffs)
    products, forms = _optimize_group(paths, n_sweeps=6)
    part_a = list(range(8))
    part_b = list(range(8, 16))

    all_sq = sorted(
        set(k[1][0] for k in products if k[0] == "pair" and k[1][0] == k[1][1])
    )
    sq_keys = set(("pair", (s, s)) for s in all_sq)

    # classify products by which groups use them
    use_a, use_b = set(), set()
    for p, form in zip(paths, forms):
        tgt = use_a if p[3] in part_a else use_b
        for r in form:
            if r and r[0] in ("sg", "pair") and r not in sq_keys:
                tgt.add(r)
    shared = use_a & use_b
    uniq = {0: use_a - shared, 1: use_b - shared}

    slot_shared, builds_shared, ns, _ = _plan_merges(shared)
    slot_a, builds_a, na, _ = _plan_merges(uniq[0])
    slot_b, builds_b, nb, _ = _plan_merges(uniq[1])
    base2 = ns
    n_main = ns + max(na, nb)
    sq_slot = {s: n_main + i for i, s in enumerate(all_sq)}
    n_slots = n_main + len(all_sq)

    def shift(builds, slot, delta):
        bs = [(b[0], b[1], b[2], b[3], b[4] + delta) for b in builds]
        sl = {k: v + delta for k, v in slot.items()}
        return bs, sl

    builds_a, slot_a = shift(builds_a, slot_a, base2)
    builds_b, slot_b = shift(builds_b, slot_b, base2)

    groups = []
    for gi, (sos, gbuilds, gslot) in enumerate(
        (
            (part_a, builds_shared + builds_a, {**slot_shared, **slot_a}),
            (part_b, builds_b, {**slot_shared, **slot_b}),
        )
    ):
        slot_of = dict(gslot)
        for s in all_sq:
            slot_of[("pair", (s, s))] = sq_slot[s]
        gidx = [i for i, p in enumerate(paths) if p[3] in sos]
        order = sorted(
            gidx,
            key=lambda i: (
                paths[i][0] != 1,
                max(
                    (
                        slot_of[r]
                        for r in forms[i]
                        if r and r[0] in ("sg", "pair")
                    ),
                    default=-1,
                ),
            ),
        )
        path_ops = [
            (paths[i][0], forms[i][0], forms[i][1], paths[i][4], paths[i][3])
            for i in order
        ]
        groups.append(
            dict(
                sos=sos,
                builds=gbuilds,
                slot_of=slot_of,
                n_slots=n_slots,
                path_ops=path_ops,
            )
        )
    return groups, all_sq


SLAB = 32  # coefficient-diagonal matrices per DMA slab


def _build_bass(groups, dtype_name, act_frac, warmup, pool_frac=0.0, all_sq=(), gpsimd_every=0):
    import concourse.bacc as bacc
    import concourse.mybir as mybir
    from concourse.tile import TileContext

    dt = mybir.dt.float32 if dtype_name == F32 else mybir.dt.bfloat16
    MULT = mybir.AluOpType.mult

    nc = bacc.Bacc("TRN2", debug=False)

    n_paths_total = sum(len(g["path_ops"]) for g in groups)
    n_slabs = (n_paths_total + SLAB - 1) // SLAB

    x1t_d = nc.dram_tensor("x1t", [S * U, ZS], dt, kind="ExternalInput")
    x0_d = nc.dram_tensor("x0w", [NELEM, S * U], dt, kind="ExternalInput")
    oh_d = nc.dram_tensor("oh", [NELEM, ZS], dt, kind="ExternalInput")
    cd_d = nc.dram_tensor("cdiag", [n_slabs * SLAB * U, U], dt, kind="ExternalInput")
    out_d = nc.dram_tensor("outt", [S * U, ZS], dt, kind="ExternalOutput")
    junk_d = nc.dram_tensor("junk", [U, ZS], mybir.dt.float32)

    max_slots = max(g["n_slots"] for g in groups)
    coeff_order = []  # flat list of coefficients in emission order

    MAXRUN = 4
    MAXSTRIDE = 63  # ISA: 16-bit step_elem field caps r-stride at 32767 elems
    TMP_BUFS = {1: 6, 2: 3, 3: 2, 4: 2}

    with TileContext(nc) as tc:
        with tc.tile_pool(name="persist", bufs=1) as persist, tc.tile_pool(
            name="tmp", bufs=6
        ) as tmp_pool, tc.tile_pool(name="slab", bufs=2) as slab_pool:
            x1t = persist.tile([U, S * ZS], dt, tag="x1t")
            x0g = persist.tile([U, S * ZS], dt, tag="x0g")
            prod = persist.tile([U, max_slots * ZS], dt, tag="prod")
            x0_sb = persist.tile([NELEM, S * U], dt, tag="x0w")
            oh_sb = persist.tile([NELEM, ZS], dt, tag="oh")
            warm_sb = persist.tile([NELEM, U], dt, tag="warmsrc")

            def seg(t, s):
                return t[:, s * ZS : (s + 1) * ZS]

            def span(t, lo, n):
                return t[:, lo * ZS : (lo + n) * ZS]

            # x1t segments first (they gate the DVE pair builds + squares),
            # in order of first use by the build schedule
            seg_order = []

            def _want(s):
                if s not in seg_order:
                    seg_order.append(s)

            for s in all_sq:
                _want(s)
            for g in groups:
                for b in g["builds"]:
                    kind, key, lo, n, _ = b
                    if kind == "pair_run":
                        for i in range(n):
                            _want(lo + i)
                            _want(lo + i + key)
                    else:
                        for i in range(n):
                            _want(lo + i)
            for s in range(S):
                _want(s)
            for s in seg_order:
                nc.sync.dma_start(out=seg(x1t, s), in_=x1t_d[s * U : (s + 1) * U, :])
            nc.sync.dma_start(out=oh_sb[:], in_=oh_d[:])
            nc.sync.dma_start(out=x0_sb[:], in_=x0_d[:])

            # global square products on ScalarE (before any ACT Copy use,
            # emitted as consecutive runs to avoid table-set thrashing)
            if all_sq:
                max_g = groups[0]["n_slots"] - len(all_sq)
                run = [all_sq[0]]
                ri = 0
                for s in list(all_sq[1:]) + [None]:
                    if s is not None and s == run[-1] + 1:
                        run.append(s)
                    else:
                        nc.scalar.activation(
                            span(prod, max_g + ri, len(run)),
                            span(x1t, run[0], len(run)),
                            mybir.ActivationFunctionType.Square,
                        )
                        ri += len(run)
                        if s is not None:
                            run = [s]

            nc.gpsimd.memset(warm_sb[:], 0.0)
            # PE warmup burst (reads a memset tile: no DMA dependency, keeps
            # the HAM clock-gate warm before the real stream starts)
            # + the 16 x0 gather matmuls
            with tc.tile_pool(name="gpsum", bufs=4, space="PSUM") as gpsum:
                if warmup > 0:
                    wt = gpsum.tile([U, ZS], mybir.dt.float32, tag="warm", bufs=1)
                    for i in range(warmup):
                        nc.tensor.matmul(
                            wt[:],
                            warm_sb[:],
                            oh_sb[:],
                            start=(i == 0),
                            stop=(i == warmup - 1),
                        )
                    ws = tmp_pool.tile(
                        [U, ZS], mybir.dt.float32, tag="warms", bufs=1
                    )
                    nc.scalar.copy(out=ws[:], in_=wt[:])
                    nc.sync.dma_start(out=junk_d[:], in_=ws[:])
                for s in range(S):
                    pt = gpsum.tile([U, ZS], mybir.dt.float32, tag="gps")
                    nc.tensor.matmul(
                        pt[:],
                        x0_sb[:, s * U : (s + 1) * U],
                        oh_sb[:],
                        start=True,
                        stop=True,
                    )
                    nc.scalar.copy(out=seg(x0g, s), in_=pt[:])

            slab_state = {"idx": -1, "tile": None}
            for g in groups:
                sos, builds, slot_of, path_ops = (
                    g["sos"],
                    g["builds"],
                    g["slot_of"],
                    g["path_ops"],
                )
                n_slots_g = g["n_slots"]
                slot_done_at = {}
                for bi, b in enumerate(builds):
                    for i in range(b[3]):
                        slot_done_at[b[4] + i] = bi

                def space_pos(r):
                    if r[0] in ("sg", "pair"):
                        return ("prod", slot_of[r])
                    if r[0] == "x1":
                        return ("x1", r[1])
                    return ("x0g", r[1])

                # ---- greedy merged-final planning ----
                # each instruction: (bcast_ref, partner_space, lo, stride, members)
                # computing tmp[:, j] = bcast (.) partner[lo + j*stride]
                finals = [i for i, po in enumerate(path_ops) if po[0] >= 2]
                d1s = [i for i, po in enumerate(path_ops) if po[0] == 1]
                unsched = set(finals)
                instrs = []
                while unsched:
                    cand_groups = {}
                    for i in unsched:
                        d, r1, r2, c, so = path_ops[i]
                        for rb, rp in ((r1, r2), (r2, r1)):
                            sp, pos = space_pos(rp)
                            cand_groups.setdefault((rb, sp), {}).setdefault(pos, i)
                    best = None
                    for (rb, sp), posmap in cand_groups.items():
                        ps = sorted(posmap)
                        for ai in range(len(ps)):
                            for bi2 in range(ai + 1, len(ps)):
                                st = ps[bi2] - ps[ai]
                                if st > MAXSTRIDE:
                                    break
                                run = [ps[ai], ps[bi2]]
                                nxt = ps[bi2] + st
                                while nxt in posmap and len(run) < MAXRUN:
                                    run.append(nxt)
                                    nxt += st
                                if best is None or len(run) > best[0]:
                                    best = (
                                        len(run),
                                        (rb, sp, run[0], st, [posmap[p] for p in run]),
                                    )
                                if best[0] >= MAXRUN:
                                    break
                            if best is not None and best[0] >= MAXRUN:
                                break
                        if best is not None and best[0] >= MAXRUN:
                            break
                    if best is None or best[0] < 2:
                        for i in sorted(unsched):
                            d, r1, r2, c, so = path_ops[i]
                            sp, pos = space_pos(r2)
                            instrs.append((r1, sp, pos, 1, [i]))
                        unsched.clear()
                    else:
                        _, ins = best
                        instrs.append(ins)
                        unsched -= set(ins[4])

                def instr_ready(ins):
                    rb, sp, lo, st, members = ins
                    bi = -1
                    if rb[0] in ("sg", "pair"):
                        bi = max(bi, slot_done_at.get(slot_of[rb], -1))
                    if sp == "prod":
                        for k in range(len(members)):
                            bi = max(bi, slot_done_at.get(lo + k * st, -1))
                    return bi

                ready_after = defaultdict(list)  # build idx -> events
                for i in d1s:
                    r1 = path_ops[i][1]
                    ready_after[slot_done_at.get(slot_of[r1], -1)].append(("d1", i))
                for j, ins in enumerate(instrs):
                    ready_after[instr_ready(ins)].append(("ins", j))

                # dry pass: MM emission order -> start/stop flags per so
                mm_seq = []
                for bi in range(-1, len(builds)):
                    for kind, j in ready_after[bi]:
                        if kind == "d1":
                            mm_seq.append(j)
                        else:
                            mm_seq.extend(instrs[j][4])
                first_for_so = {}
                last_for_so = {}
                for i in mm_seq:
                    so = path_ops[i][4]
                    if so not in first_for_so:
                        first_for_so[so] = i
                    last_for_so[so] = i
                assert len(mm_seq) == len(path_ops)

                acc = {}
                with tc.tile_pool(
                    name=f"acc{sos[0]}", bufs=8, space="PSUM"
                ) as acc_pool:
                    for so in sos:
                        if so in first_for_so:
                            acc[so] = acc_pool.tile(
                                [U, ZS],
                                mybir.dt.float32,
                                tag=f"acc{sos.index(so)}",
                                name=f"acc_{so}",
                                bufs=1,
                            )

                    def pref(r):
                        kind, key = r
                        if kind == "x1":
                            return seg(x1t, key)
                        if kind == "x0g":
                            return seg(x0g, key)
                        return seg(prod, slot_of[r])

                    def emit_mm(i, rhs):
                        d, r1, r2, c, so = path_ops[i]
                        gi = len(coeff_order)
                        coeff_order.append(c)
                        sj, sk = gi // SLAB, gi % SLAB
                        if slab_state["idx"] != sj:
                            slab_state["idx"] = sj
                            stt = slab_pool.tile(
                                [U, SLAB * U], dt, tag="slab", name=f"slab{sj}"
                            )
                            slab_state["tile"] = stt
                            nc.sync.dma_start(
                                out=stt[:].rearrange("p (d c) -> p d c", d=SLAB),
                                in_=cd_d[sj * SLAB * U : (sj + 1) * SLAB * U, :]
                                .rearrange("(d p) c -> p d c", p=U),
                            )
                        stt = slab_state["tile"]
                        nc.tensor.matmul(
                            acc[so][:],
                            stt[:, sk * U : (sk + 1) * U],
                            rhs,
                            start=(i == first_for_so[so]),
                            stop=(i == last_for_so[so]),
                        )

                    base_of = {"prod": (prod, max_slots), "x1": (x1t, S), "x0g": (x0g, S)}

                    def emit_instr(j):
                        rb, sp, lo, st, members = instrs[j]
                        n = len(members)
                        base, W = base_of[sp]
                        if n == 1:
                            t1 = tmp_pool.tile(
                                [U, ZS], dt, tag="tmp1", bufs=TMP_BUFS[1],
                                name=f"t{sos[0]}_{j}",
                            )
                            nc.vector.tensor_tensor(
                                out=t1[:], in0=pref(rb), in1=seg(base, lo), op=MULT
                            )
                        else:
                            t1 = tmp_pool.tile(
                                [U, n * ZS], dt, tag=f"tmp{n}", bufs=TMP_BUFS[n],
                                name=f"t{sos[0]}_{j}",
                            )
                            in0 = (
                                pref(rb)
                                .rearrange("p (o z) -> p o z", o=1)
                                .broadcast_to([U, n, ZS])
                            )
                            base3 = base[:].rearrange("p (w z) -> p w z", w=W)
                            in1 = base3[:, lo : lo + (n - 1) * st + 1 : st, :]
                            out3 = t1[:].rearrange("p (r z) -> p r z", r=n)
                            nc.vector.tensor_tensor(
                                out=out3, in0=in0, in1=in1, op=MULT
                            )
                        for k, i in enumerate(members):
                            emit_mm(i, t1[:, k * ZS : (k + 1) * ZS])

                    def emit_event(ev):
                        kind, j = ev
                        if kind == "d1":
                            emit_mm(j, pref(path_ops[j][1]))
                        else:
                            emit_instr(j)

                    for ev in ready_after[-1]:
                        emit_event(ev)
                    for bi, b in enumerate(builds):
                        kind = b[0]
                        if kind == "sg_run":
                            _, s0, s_lo, n, slot_lo = b
                            in0 = (
                                seg(x0g, s0)
                                .rearrange("p (o z) -> p o z", o=1)
                                .broadcast_to([U, n, ZS])
                            )
                            in1 = span(x1t, s_lo, n).rearrange(
                                "p (r z) -> p r z", r=n
                            )
                            out = span(prod, slot_lo, n).rearrange(
                                "p (r z) -> p r z", r=n
                            )
                            nc.vector.tensor_tensor(
                                out=out, in0=in0, in1=in1, op=MULT
                            )
                        else:
                            _, delta, a_lo, n, slot_lo = b
                            in0 = span(x1t, a_lo, n).rearrange(
                                "p (r z) -> p r z", r=n
                            )
                            in1 = span(x1t, a_lo + delta, n).rearrange(
                                "p (r z) -> p r z", r=n
                            )
                            out = span(prod, slot_lo, n).rearrange(
                                "p (r z) -> p r z", r=n
                            )
                            nc.vector.tensor_tensor(
                                out=out, in0=in0, in1=in1, op=MULT
                            )
                        for ev in ready_after[bi]:
                            emit_event(ev)

                    # drain each accumulator through a small SBUF stage
                    for so in sos:
                        assert so in acc, f"output segment {so} has no paths"
                        ostg = tmp_pool.tile(
                            [U, ZS], dt, tag="ostg", bufs=4, name=f"ostg{so}"
                        )
                        nc.scalar.copy(out=ostg[:], in_=acc[so][:])
                        nc.sync.dma_start(
                            out=out_d[so * U : (so + 1) * U, :], in_=ostg[:]
                        )

    nc.compile()
    return nc, coeff_order


def kernel(x0, x1, coeff1, coeff2, coeff3, i0, idx1, idx2, idx3):
    global LAST_EXEC_NS, LAST_RESULTS
    from concourse.bass_utils import run_bass_kernel_spmd

    x0 = np.asarray(x0, dtype=np.float32)
    x1 = np.asarray(x1, dtype=np.float32)
    i0 = np.asarray(i0).astype(np.int64)
    idxs = [np.asarray(a) for a in (idx1, idx2, idx3)]
    coeffs = [np.asarray(c, dtype=np.float32) for c in (coeff1, coeff2, coeff3)]

    dtype_name = os.environ.get("KERNEL_DTYPE", "bfloat16")
    act_frac = float(os.environ.get("KERNEL_ACT_FRAC", "0.55"))
    pool_frac = float(os.environ.get("KERNEL_POOL_FRAC", "0.3"))
    warmup = int(os.environ.get("KERNEL_WARMUP", "12"))
    gpsimd_every = int(os.environ.get("KERNEL_GPSIMD_EVERY", "0"))
    npdt = np.float32
    if dtype_name != F32:
        import ml_dtypes

        npdt = ml_dtypes.bfloat16

    groups, all_sq = _build_plan(idxs, coeffs)
    nc, coeff_order = _build_bass(groups, dtype_name, act_frac, warmup, pool_frac, all_sq, gpsimd_every)
    n_slabs = (len(coeff_order) + SLAB - 1) // SLAB
    cdiag = np.zeros((n_slabs * SLAB * U, U), dtype=npdt)
    for gi, c in enumerate(coeff_order):
        blk = cdiag[gi * U : (gi + 1) * U, :]
        np.fill_diagonal(blk, np.asarray(c, dtype=npdt))

    in_maps = []
    eye = np.arange(NELEM)
    x0c = x0.astype(npdt)
    for c in range(NCORES):
        zl, zh = c * ZS, (c + 1) * ZS
        shard = x1[zl:zh]
        x1t = np.ascontiguousarray(
            shard.reshape(ZS, S, U).transpose(1, 2, 0).reshape(S * U, ZS)
        ).astype(npdt)
        oh = (i0[zl:zh][None, :] == eye[:, None]).astype(npdt)
        in_maps.append({"x1t": x1t, "x0w": x0c, "oh": oh, "cdiag": cdiag})

    trace = os.environ.get("BASS_TRACE", "") not in ("", "0")
    trace_cores = None
    tc_env = os.environ.get("KERNEL_TRACE_CORES", "")
    if tc_env:
        trace_cores = [int(x) for x in tc_env.split(",")]
    res = run_bass_kernel_spmd(
        nc, in_maps, core_ids=list(range(NCORES)), trace=trace,
        trace_cores=trace_cores,
    )
    LAST_EXEC_NS = res.exec_time_ns
    LAST_RESULTS = res

    out = np.empty((Z, S * U), dtype=np.float32)
    for c in range(NCORES):
        outt = np.asarray(res.results[c]["outt"], dtype=np.float32)
        out[c * ZS : (c + 1) * ZS] = (
            outt.reshape(S, U, ZS).transpose(2, 0, 1).reshape(ZS, S * U)
        )
    return out

